# revision 37
# baseline (speedup 1.0000x reference)
"""Mamba block (MockMambaBlock) on 8 Trainium2 NeuronCores.

Sharding: tensor-parallel over d_inner (8 x 256 channels), both batches on
every core. The x_proj/dt_proj contraction over d_inner is completed with an
on-device AllReduce of the small (32, L) partial per batch; out_proj
row-partials are summed on the host (the gather step).

Schedule: phase A computes the in_proj x-half (conv + x_proj) of batch 0
first so its AllReduce fires early; the z-half and batch 1 follow. The SSM
scans (the DVE wall: 2 cycles/element regardless of dtype) start as soon as
md(b0) lands and overlap the rest of phase A. The u = dtx*B_n multiplies stay on
the DVE (GpSimd shares SBUF ports with the DVE, so offloading them slows the
scans more than it saves); exp/silu/sigmoid and the PSUM->SBUF drains run on
the scalar engine, interleaved so the scan supply chain never starves. out_proj is emitted per 128-token tile right behind the
gating to keep the tail short. Issue order doubles as per-engine program
order, so hooks thread low-priority work into the busy streams.
"""

import sys

sys.path.insert(0, "/opt/trn_rl_repo")

import numpy as np
import ml_dtypes

import concourse.bass as bass
import concourse.bacc as bacc
import concourse.mybir as mybir
import concourse.tile as tile
from concourse.bass_utils import run_bass_kernel_spmd

F32 = mybir.dt.float32
BF16 = mybir.dt.bfloat16
AF = mybir.ActivationFunctionType
OP = mybir.AluOpType

B, L, DM, DI, DS, DC = 2, 2048, 1024, 2048, 16, 4
NCORES = 8
DIL = DI // NCORES          # 256 channels per core
NBLK = DIL // 128           # 2 partition blocks of channels
KBLK = DM // 128            # 8 contraction blocks for in_proj
LTA = 512                   # token chunk
NCH = L // LTA              # 4 chunks
NPT = L // 512              # psum tiles per row


def build_nc():
    nc = bacc.Bacc()

    x_t = nc.dram_tensor("x_t", [B, KBLK, 128, L], BF16, kind="ExternalInput")
    win_d = nc.dram_tensor("win", [DM, 2 * DIL], BF16, kind="ExternalInput")
    wout_d = nc.dram_tensor("wout", [DIL, DM], BF16, kind="ExternalInput")
    wx_d = nc.dram_tensor("wx", [DIL, 2 * DS], BF16, kind="ExternalInput")
    wdt_d = nc.dram_tensor("wdt", [DS, DIL], BF16, kind="ExternalInput")
    a_d = nc.dram_tensor("a", [DIL, DS], F32, kind="ExternalInput")
    convw_d = nc.dram_tensor("convw", [DIL, DC], F32, kind="ExternalInput")
    convb_d = nc.dram_tensor("convb", [DIL, 1], F32, kind="ExternalInput")
    dvec_d = nc.dram_tensor("dvec", [DIL, 1], F32, kind="ExternalInput")
    bdt_d = nc.dram_tensor("bdt", [DIL, 1], F32, kind="ExternalInput")
    identb_d = nc.dram_tensor("identb", [128, 128], BF16, kind="ExternalInput")
    diagd_d = nc.dram_tensor("diagd", [DIL, 128], BF16, kind="ExternalInput")
    out_d = nc.dram_tensor("out_p", [B, L, DM], BF16, kind="ExternalOutput")

    with tile.TileContext(nc) as tc:
        with (
            tc.tile_pool(name="weights", bufs=1) as wp,
            tc.tile_pool(name="resident", bufs=1) as rp,
            tc.tile_pool(name="dram", bufs=1, space="DRAM") as dp,
        ):
            # ---- weights to SBUF ----
            # group 1: what x_branch(0) touches; the rest is issued after its
            # x loads so the first in_proj matmul starts sooner
            win_sb = wp.tile([128, KBLK, 2 * DIL], BF16)
            nc.sync.dma_start(win_sb[:], win_d[:].rearrange("(k p) m -> p k m", p=128))
            wx_sb = wp.tile([128, NBLK, 2 * DS], BF16)
            nc.sync.dma_start(wx_sb[:], wx_d[:].rearrange("(k p) m -> p k m", p=128))
            convw_sb = wp.tile([128, NBLK, DC], F32)
            nc.sync.dma_start(convw_sb[:], convw_d[:].rearrange("(k p) m -> p k m", p=128))
            convb_sb = wp.tile([128, NBLK, 1], F32)
            nc.sync.dma_start(convb_sb[:], convb_d[:].rearrange("(k p) m -> p k m", p=128))
            wout_sb = wp.tile([128, NBLK, DM], BF16)
            wdt_sb = wp.tile([DS, DIL], BF16)
            a_sb = wp.tile([128, NBLK, DS], F32)
            dvec_sb = wp.tile([128, NBLK, 1], F32)
            bdt_sb = wp.tile([128, NBLK, 1], F32)
            identb_sb = wp.tile([128, 128], BF16)
            diagd_sb = wp.tile([128, NBLK, 128], BF16)

            def load_weights_group2():
                nc.sync.dma_start(wout_sb[:], wout_d[:].rearrange("(k p) m -> p k m", p=128))
                nc.sync.dma_start(wdt_sb[:], wdt_d[:])
                nc.sync.dma_start(a_sb[:], a_d[:].rearrange("(k p) m -> p k m", p=128))
                nc.sync.dma_start(dvec_sb[:], dvec_d[:].rearrange("(k p) m -> p k m", p=128))
                nc.sync.dma_start(bdt_sb[:], bdt_d[:].rearrange("(k p) m -> p k m", p=128))
                nc.sync.dma_start(identb_sb[:], identb_d[:])
                nc.sync.dma_start(diagd_sb[:], diagd_d[:].rearrange("(k p) m -> p k m", p=128))

            # ---- resident activations ----
            xcv = [[rp.tile([128, L], BF16, name=f"xcv{b_}{k}", tag=f"xcv{b_}{k}")
                    for k in range(NBLK)] for b_ in range(B)]
            zac = [[rp.tile([128, L], BF16, name=f"zac{b_}{k}", tag=f"zac{b_}{k}")
                    for k in range(NBLK)] for b_ in range(B)]
            # md holds -dt; after the scans it is overwritten in place by the
            # gated ssm output (yin) to save SBUF.
            md = [[rp.tile([128, L], BF16, name=f"md{b_}{k}", tag=f"md{b_}{k}")
                   for k in range(NBLK)] for b_ in range(B)]
            dtin_sb = [rp.tile([DS, L], BF16, name=f"dtin{b_}", tag=f"dtin{b_}")
                       for b_ in range(B)]
            xs_sb = [rp.tile([2 * DS, L], BF16, name=f"xs{b_}", tag=f"xs{b_}")
                     for b_ in range(B)]
            xp = [rp.tile([128, LTA + DC - 1], BF16, name=f"xp{k}", tag=f"xp{k}")
                  for k in range(NBLK)]

            cc_in_dt = [dp.tile([DS, L], BF16, name=f"cc_in_dt{b_}") for b_ in range(B)]
            cc_in_B = [dp.tile([DS, L], BF16, name=f"cc_in_B{b_}") for b_ in range(B)]
            cc_out_dt = [dp.tile([DS, L], BF16, addr_space="Shared",
                                 name=f"cc_out_dt{b_}") for b_ in range(B)]
            cc_out_B = [dp.tile([DS, L], BF16, addr_space="Shared",
                                name=f"cc_out_B{b_}") for b_ in range(B)]

            with (
                tc.tile_pool(name="yps", bufs=1, space="PSUM") as ypsp,
                tc.tile_pool(name="pax", bufs=1) as pax,
                tc.tile_pool(name="pb", bufs=1) as pb,
            ):
                dtpsp_cm = tc.tile_pool(name="dtps", bufs=2, space="PSUM")
                dtpsp = dtpsp_cm.__enter__()
                # ---------------- phase A helpers ----------------
                def x_branch(b_, paps, mid_hook=None):
                    xsx_tiles = []
                    for ch in range(NCH):
                        t0 = ch * LTA
                        xsx = pax.tile([128, KBLK, LTA], BF16, tag="xsx", bufs=4,
                                       name=f"xsx{b_}{ch}")
                        nc.sync.dma_start(
                            xsx[:], x_t[b_].transpose([1, 0, 2])[:, :, t0:t0 + LTA])
                        xsx_tiles.append(xsx)
                    for ch in range(NCH):
                        t0 = ch * LTA
                        xsx = xsx_tiles[ch]
                        for blk in range(NBLK):
                            ps = paps.tile([128, LTA], F32, tag="ps_in",
                                           name=f"psx{b_}{ch}{blk}")
                            for kb in range(KBLK):
                                nc.tensor.matmul(
                                    ps[:],
                                    win_sb[:, kb, blk * 128:(blk + 1) * 128],
                                    xsx[:, kb, :],
                                    start=(kb == 0), stop=(kb == KBLK - 1))
                            # depthwise causal conv, 4 taps, bf16
                            if ch == 0:
                                nc.vector.memset(xp[blk][:, 0:DC - 1], 0.0)
                            else:
                                nc.vector.tensor_copy(
                                    xp[blk][:, 0:DC - 1],
                                    xp[blk][:, LTA:LTA + DC - 1])
                            nc.scalar.copy(xp[blk][:, DC - 1:LTA + DC - 1], ps[:])
                            c0 = pax.tile([128, LTA], BF16, tag="cv0", bufs=2,
                                          name=f"c0{b_}{ch}{blk}")
                            c1 = pax.tile([128, LTA], BF16, tag="cv1", bufs=2,
                                          name=f"c1{b_}{ch}{blk}")
                            c2 = pax.tile([128, LTA], BF16, tag="cv2", bufs=2,
                                          name=f"c2{b_}{ch}{blk}")
                            nc.vector.tensor_scalar_mul(
                                c0[:], xp[blk][:, 0:LTA], convw_sb[:, blk, 0:1])
                            nc.vector.tensor_scalar_mul(
                                c1[:], xp[blk][:, 1:1 + LTA], convw_sb[:, blk, 1:2])
                            nc.vector.tensor_add(c0[:], c0[:], c1[:])
                            nc.vector.tensor_scalar_mul(
                                c1[:], xp[blk][:, 2:2 + LTA], convw_sb[:, blk, 2:3])
                            nc.vector.tensor_scalar_mul(
                                c2[:], xp[blk][:, 3:3 + LTA], convw_sb[:, blk, 3:4])
                            nc.vector.tensor_add(c1[:], c1[:], c2[:])
                            nc.vector.tensor_add(c0[:], c0[:], c1[:])
                            nc.scalar.activation(
                                xcv[b_][blk][:, t0:t0 + LTA], c0[:],
                                AF.Silu, bias=convb_sb[:, blk, :])
                        # x_proj partial for this chunk (borrows a ps_in slot)
                        psx = paps.tile([128, LTA], F32, tag="ps_in",
                                        name=f"psxp{b_}{ch}")
                        for kb in range(NBLK):
                            nc.tensor.matmul(
                                psx[0:2 * DS, :], wx_sb[:, kb, :],
                                xcv[b_][kb][:, t0:t0 + LTA],
                                start=(kb == 0), stop=(kb == NBLK - 1))
                        nc.scalar.copy(xs_sb[b_][:, t0:t0 + LTA], psx[0:2 * DS, :])
                        if ch == 2 and mid_hook is not None:
                            mid_hook()
                    return xsx_tiles

                def z_mms(b_, ch, xsz, paps):
                    outs = []
                    for blk in range(NBLK):
                        ps = paps.tile([128, LTA], F32, tag="ps_in",
                                       name=f"psz{b_}{ch}{blk}")
                        for kb in range(KBLK):
                            nc.tensor.matmul(
                                ps[:],
                                win_sb[:, kb, (NBLK + blk) * 128:(NBLK + blk + 1) * 128],
                                xsz[:, kb, :],
                                start=(kb == 0), stop=(kb == KBLK - 1))
                        outs.append((blk, ps))
                    return outs

                def z_silu(b_, ch, outs):
                    t0 = ch * LTA
                    for blk, ps in outs:
                        nc.scalar.activation(
                            zac[b_][blk][:, t0:t0 + LTA], ps[:], AF.Silu)

                def dt_half(b_, blk):
                    # md = -softplus(dt_raw + b_dt) = ln(sigmoid(-(dt_raw + b_dt)))
                    for ch in range(NCH):
                        t0 = ch * LTA
                        psd = dtpsp.tile([128, LTA], F32, tag="psd",
                                         name=f"psd{b_}{ch}{blk}")
                        nc.tensor.matmul(
                            psd[:], wdt_sb[:, blk * 128:(blk + 1) * 128],
                            dtin_sb[b_][:, t0:t0 + LTA],
                            start=True, stop=True)
                        nc.scalar.activation(
                            md[b_][blk][:, t0:t0 + LTA], psd[:],
                            AF.Sigmoid, bias=bdt_sb[:, blk, :], scale=-1.0)
                    nc.scalar.activation(md[b_][blk][:], md[b_][blk][:], AF.Ln)

                def issue_cc_dma(b_):
                    nc.sync.dma_start(cc_in_dt[b_][:], xs_sb[b_][0:DS, :])
                    nc.sync.dma_start(cc_in_B[b_][:], xs_sb[b_][DS:2 * DS, :])

                def issue_cc_trigger(b_):
                    # dt rows first: the md chain is the critical path
                    nc.gpsimd.collective_compute(
                        "AllReduce", OP.add,
                        ins=[cc_in_dt[b_].opt()], outs=[cc_out_dt[b_].opt()],
                        replica_groups=[list(range(NCORES))])
                    nc.gpsimd.collective_compute(
                        "AllReduce", OP.add,
                        ins=[cc_in_B[b_].opt()], outs=[cc_out_B[b_].opt()],
                        replica_groups=[list(range(NCORES))])

                def issue_bb(b_, blk, p, nsplit=2):
                    # one tile per PAIR of state rows (2p, 2p+1); the split
                    # broadcasts land on several DMA engines in parallel,
                    # cutting the 512KB replication latency
                    bb2 = pb.tile([128, 2, L], BF16, tag="bbn", bufs=2,
                                  name=f"bb{b_}{blk}{p}")
                    h = L // nsplit
                    for j in range(2):
                        for k in range(nsplit):
                            nc.sync.dma_start(
                                bb2[:, j, k * h:(k + 1) * h],
                                cc_out_B[b_][2 * p + j:2 * p + j + 1,
                                             k * h:(k + 1) * h]
                                .broadcast_to([128, h]))
                    return bb2

                # ---------------- phase B helpers ----------------
                _da_zeroed = [0]

                def ssm_block(b_, blk, pre_bb=(), scalar_hook=None):
                    """scans + y accumulation for one (batch, blk). Hooks let
                    us interleave other work into the scalar/gpsimd streams."""
                    dtx = pb.tile([128, L], BF16, tag="dtx", bufs=1,
                                  name=f"dtx{b_}{blk}")
                    nc.vector.tensor_mul(dtx[:], md[b_][blk][:], xcv[b_][blk][:])
                    y_ps = [ypsp.tile([128, 512], F32, tag=f"yps{pt}",
                                      name=f"yps{b_}{blk}{pt}")
                            for pt in range(NPT)]
                    h2 = None
                    for n in range(DS):
                        p, j = n // 2, n % 2
                        if j == 0:
                            bb2 = (pre_bb[p] if p < len(pre_bb)
                                   else issue_bb(b_, blk, p))
                            # da for the PAIR; flat col 2048 (second half t=0)
                            # is a permanent zero -> the merged scan's state
                            # resets exactly at the pair boundary
                            da2 = pb.tile([128, 2, L], F32, tag="dan", bufs=2,
                                          name=f"da{b_}{blk}{p}")
                            if _da_zeroed[0] < 2:
                                nc.vector.memset(da2[:, 1, 0:1], 0.0)
                                _da_zeroed[0] += 1
                            nc.scalar.activation(
                                da2[:, 0, :], md[b_][blk][:], AF.Exp,
                                scale=a_sb[:, blk, n:n + 1])
                            nc.scalar.activation(
                                da2[:, 1, 1:L], md[b_][blk][:, 1:L], AF.Exp,
                                scale=a_sb[:, blk, n + 1:n + 2])
                            u2 = pb.tile([128, 2, L], BF16, tag="un", bufs=2,
                                         name=f"u{b_}{blk}{p}")
                            nc.vector.tensor_mul(
                                u2[:],
                                dtx[:].rearrange("q (o l) -> q o l", o=1)
                                .broadcast_to([128, 2, L]),
                                bb2[:])
                            h2 = pb.tile([128, 2, L], BF16, tag="hn", bufs=2,
                                         name=f"h{b_}{blk}{p}")
                            nc.vector.tensor_tensor_scan(
                                h2[:].rearrange("q o l -> q (o l)"),
                                da2[:].rearrange("q o l -> q (o l)"),
                                u2[:].rearrange("q o l -> q (o l)"),
                                0.0, OP.mult, OP.add)
                        if scalar_hook is not None:
                            scalar_hook(n)
                        for pt in range(NPT):
                            nc.tensor.matmul(
                                y_ps[pt][:], identb_sb[:],
                                h2[:, j, pt * 512:(pt + 1) * 512],
                                start=(n == 0), stop=False)
                    return y_ps

                def gate_block(b_, blk, y_ps, per_pt=None):
                    yin = md[b_][blk]
                    for pt in range(NPT):
                        nc.tensor.matmul(
                            y_ps[pt][:], diagd_sb[:, blk, :],
                            xcv[b_][blk][:, pt * 512:(pt + 1) * 512],
                            start=False, stop=True)
                        nc.vector.tensor_mul(
                            yin[:, pt * 512:(pt + 1) * 512], y_ps[pt][:],
                            zac[b_][blk][:, pt * 512:(pt + 1) * 512])
                        if per_pt is not None:
                            per_pt(pt)
                    return yin

                def out_proj_mt(b_, yins, mt, dmh, psop):
                    ps_o = psop.tile([128, 512], F32, tag="ps_o", bufs=2,
                                     name=f"pso{b_}{mt}{dmh}")
                    for blk in range(NBLK):
                        nc.tensor.matmul(
                            ps_o[:],
                            yins[blk][:, mt * 128:(mt + 1) * 128],
                            wout_sb[:, blk, dmh * 512:(dmh + 1) * 512],
                            start=(blk == 0), stop=(blk == NBLK - 1))
                    osb = pb.tile([128, 512], BF16, tag="osb", bufs=4,
                                  name=f"osb{b_}{mt}{dmh}")
                    nc.scalar.copy(osb[:], ps_o[:])
                    nc.sync.dma_start(
                        out_d[b_, mt * 128:(mt + 1) * 128,
                              dmh * 512:(dmh + 1) * 512],
                        osb[:])

                # ================= issue: phase A =================
                if True:
                    paps_cm = tc.tile_pool(name="paps", bufs=2, space="PSUM")
                    paps = paps_cm.__enter__()
                    xb0_tiles = x_branch(0, paps)
                    load_weights_group2()
                    issue_cc_dma(0)
                    issue_cc_trigger(0)
                    for ch in range(NCH):
                        z_silu(0, ch, z_mms(0, ch, xb0_tiles[ch], paps))

                    pre_bb00 = []

                    def mid_b1():
                        # dt(b0) threaded mid-x_branch(1): x loads are already
                        # hoisted, so the AllReduce-gated DMAs blocking the
                        # sync queue delay nothing urgent behind them
                        nc.sync.dma_start(dtin_sb[0][:], cc_out_dt[0][:])
                        pre_bb00.append(issue_bb(0, 0, 0, nsplit=4))
                        pre_bb00.append(issue_bb(0, 0, 1))
                        dt_half(0, 0)

                    xb1_tiles = x_branch(1, paps)
                    issue_cc_dma(1)
                    issue_cc_trigger(1)
                    mid_b1()

                # ================= issue: phase B =================
                if True:
                    # z(b1) and dt(b0,blk1) are issued inside block 0: the
                    # engine-progress semaphores make anything issued before
                    # the ln/exp chain delay it, so all non-critical work goes
                    # after the first exps
                    def sc_hook_00(n):
                        if n == 11:
                            dt_half(0, 1)

                    yps00 = ssm_block(0, 0, pre_bb=pre_bb00,
                                      scalar_hook=sc_hook_00)

                    # dt(b1) blk0: tensor work lands after identity(b0,blk0)
                    nc.sync.dma_start(dtin_sb[1][:], cc_out_dt[1][:])
                    dt_half(1, 0)
                    yin00 = gate_block(0, 0, yps00)

                    # z(b1) hosted in block 1: block 0's broadcast supply is
                    # still catching up from the AllReduce and runs hotter
                    def sc_hook_01(n):
                        if n in (3, 9):
                            for ch in (0, 1) if n == 3 else (2, 3):
                                z_silu(1, ch, z_mms(1, ch, xb1_tiles[ch], paps))

                    yps01 = ssm_block(0, 1, scalar_hook=sc_hook_01)
                    paps_cm.__exit__(None, None, None)
                    dt_half(1, 1)
                    dtpsp_cm.__exit__(None, None, None)
                    psop_cm = tc.tile_pool(name="pso", bufs=4, space="PSUM")
                    psop = psop_cm.__enter__()
                    yin01 = gate_block(0, 1, yps01)

                    # (b1, blk0): interleave out_proj(b0) behind the exps so
                    # the scalar engine drains PSUM without starving the scans
                    op0 = [(mt, dmh) for mt in range(L // 128) for dmh in range(2)]

                    def op_hook_b1(n):
                        for j in range(2):
                            idx = n * 2 + j
                            if idx < len(op0):
                                mt, dmh = op0[idx]
                                out_proj_mt(0, [yin00, yin01], mt, dmh, psop)

                    yps10 = ssm_block(1, 0, scalar_hook=op_hook_b1)
                    yin10 = gate_block(1, 0, yps10)

                    # out_proj(b1): the blk0 half-products for token tiles
                    # mt=4..15 are computed into spare bf16 buffers (dead b0
                    # tiles) while (b1,blk1) scans run, so the tail after the
                    # last scan holds only the blk1 matmuls + cheap adds.
                    obuf_tiles = [xcv[0][0], xcv[0][1], zac[0][0], zac[0][1],
                                  yin00, yin01]

                    def obuf_slot(idx):
                        t = obuf_tiles[(idx - 8) // 4]
                        c = ((idx - 8) % 4) * 512
                        return t[:, c:c + 512]

                    def half0_hook(n):
                        # 2 groups per n covers idx 8..31 over n=0..11
                        for j in range(2):
                            idx = 8 + n * 2 + j
                            if idx < 32:
                                mt, dmh = idx // 2, idx % 2
                                ps_h = psop.tile([128, 512], F32, tag="ps_o",
                                                 bufs=2, name=f"ph{mt}{dmh}")
                                nc.tensor.matmul(
                                    ps_h[:],
                                    yin10[:, mt * 128:(mt + 1) * 128],
                                    wout_sb[:, 0, dmh * 512:(dmh + 1) * 512],
                                    start=True, stop=True)
                                nc.scalar.copy(obuf_slot(idx), ps_h[:])

                    yps11 = ssm_block(1, 1, scalar_hook=half0_hook)

                    # gate blk1 per-pt and chase it with the out_proj(b1) tail
                    def op1_pt(pt):
                        for mt in range(pt * 4, pt * 4 + 4):
                            for dmh in range(2):
                                idx = mt * 2 + dmh
                                if idx < 8:
                                    out_proj_mt(1, [yin10, md[1][1]], mt, dmh,
                                                psop)
                                    continue
                                ps_t = psop.tile([128, 512], F32, tag="ps_o",
                                                 bufs=2, name=f"pt{mt}{dmh}")
                                nc.tensor.matmul(
                                    ps_t[:],
                                    md[1][1][:, mt * 128:(mt + 1) * 128],
                                    wout_sb[:, 1, dmh * 512:(dmh + 1) * 512],
                                    start=True, stop=True)
                                osb = pb.tile([128, 512], BF16, tag="osb", bufs=4,
                                              name=f"osbt{mt}{dmh}")
                                nc.vector.tensor_add(osb[:], ps_t[:],
                                                     obuf_slot(idx))
                                eng = nc.sync if (mt + dmh) % 2 == 0 else nc.scalar
                                eng.dma_start(
                                    out_d[1, mt * 128:(mt + 1) * 128,
                                          dmh * 512:(dmh + 1) * 512],
                                    osb[:])

                    gate_block(1, 1, yps11, per_pt=op1_pt)
                    psop_cm.__exit__(None, None, None)

    nc.compile()
    return nc


_NC_CACHE = {}


def _get_nc():
    if "nc" not in _NC_CACHE:
        _NC_CACHE["nc"] = build_nc()
    return _NC_CACHE["nc"]


def make_in_maps(x, W_in, conv_w, conv_b, W_x, W_dt, b_dt, A_log, D, W_out):
    x = np.asarray(x, np.float32)
    W_in = np.asarray(W_in, np.float32)
    conv_w = np.asarray(conv_w, np.float32)
    conv_b = np.asarray(conv_b, np.float32)
    W_x = np.asarray(W_x, np.float32)
    W_dt = np.asarray(W_dt, np.float32)
    b_dt = np.asarray(b_dt, np.float32)
    A_log = np.asarray(A_log, np.float32)
    D = np.asarray(D, np.float32)
    W_out = np.asarray(W_out, np.float32)

    xt = np.ascontiguousarray(x.transpose(0, 2, 1)).reshape(B, KBLK, 128, L).astype(ml_dtypes.bfloat16)
    A = np.exp(A_log)  # |A|; device uses a = -|A|, md = +dt

    in_maps = []
    for c in range(NCORES):
        lo = c * DIL
        sl = slice(lo, lo + DIL)
        in_maps.append({
            "x_t": xt,
            "win": np.ascontiguousarray(
                np.concatenate([W_in[:, sl], W_in[:, DI + lo:DI + lo + DIL]],
                               axis=1)).astype(ml_dtypes.bfloat16),
            "wout": np.ascontiguousarray(W_out[sl]).astype(ml_dtypes.bfloat16),
            "wx": np.ascontiguousarray(
                np.concatenate([W_x[sl, :DS], -W_x[sl, DS:]], axis=1)
            ).astype(ml_dtypes.bfloat16),
            "wdt": np.ascontiguousarray(W_dt[:, sl]).astype(ml_dtypes.bfloat16),
            "a": np.ascontiguousarray(A[sl]),
            "convw": np.ascontiguousarray(conv_w[sl]),
            "convb": np.ascontiguousarray(conv_b[sl, None]),
            "dvec": np.ascontiguousarray(D[sl, None]),
            "bdt": np.ascontiguousarray(-b_dt[sl, None]),
            "identb": np.eye(128, dtype=ml_dtypes.bfloat16),
            "diagd": np.stack([np.diag(D[lo + k * 128:lo + (k + 1) * 128])
                               for k in range(NBLK)]).reshape(DIL, 128)
                       .astype(ml_dtypes.bfloat16),
        })
    return in_maps


def kernel(**inputs):
    nc = _get_nc()
    in_maps = make_in_maps(**inputs)
    res = run_bass_kernel_spmd(nc, in_maps, list(range(NCORES)))
    out = np.zeros((B, L, DM), np.float32)
    for c in range(NCORES):
        out += np.asarray(res.results[c]["out_p"], dtype=np.float32)
    return out


# revision 38
# speedup vs baseline: 1.0116x; 1.0116x over previous
"""Mamba block (MockMambaBlock) on 8 Trainium2 NeuronCores.

Sharding: tensor-parallel over d_inner (8 x 256 channels), both batches on
every core. The x_proj/dt_proj contraction over d_inner is completed with an
on-device AllReduce of the small (32, L) partial per batch; out_proj
row-partials are summed on the host (the gather step).

Schedule: phase A computes the in_proj x-half (conv + x_proj) of batch 0
first so its AllReduce fires early; the z-half and batch 1 follow. The SSM
scans (the DVE wall: 2 cycles/element regardless of dtype) start as soon as
md(b0) lands and overlap the rest of phase A. The u = dtx*B_n multiplies stay on
the DVE (GpSimd shares SBUF ports with the DVE, so offloading them slows the
scans more than it saves); exp/silu/sigmoid and the PSUM->SBUF drains run on
the scalar engine, interleaved so the scan supply chain never starves. out_proj is emitted per 128-token tile right behind the
gating to keep the tail short. Issue order doubles as per-engine program
order, so hooks thread low-priority work into the busy streams.
"""

import sys

sys.path.insert(0, "/opt/trn_rl_repo")

import numpy as np
import ml_dtypes

import concourse.bass as bass
import concourse.bacc as bacc
import concourse.mybir as mybir
import concourse.tile as tile
from concourse.bass_utils import run_bass_kernel_spmd

F32 = mybir.dt.float32
BF16 = mybir.dt.bfloat16
AF = mybir.ActivationFunctionType
OP = mybir.AluOpType

B, L, DM, DI, DS, DC = 2, 2048, 1024, 2048, 16, 4
NCORES = 8
DIL = DI // NCORES          # 256 channels per core
NBLK = DIL // 128           # 2 partition blocks of channels
KBLK = DM // 128            # 8 contraction blocks for in_proj
LTA = 512                   # token chunk
NCH = L // LTA              # 4 chunks
NPT = L // 512              # psum tiles per row


def build_nc():
    nc = bacc.Bacc()

    x_t = nc.dram_tensor("x_t", [B, KBLK, 128, L], BF16, kind="ExternalInput")
    win_d = nc.dram_tensor("win", [DM, 2 * DIL], BF16, kind="ExternalInput")
    wout_d = nc.dram_tensor("wout", [DIL, DM], BF16, kind="ExternalInput")
    wx_d = nc.dram_tensor("wx", [DIL, 2 * DS], BF16, kind="ExternalInput")
    wdt_d = nc.dram_tensor("wdt", [DS, DIL], BF16, kind="ExternalInput")
    a_d = nc.dram_tensor("a", [DIL, DS], F32, kind="ExternalInput")
    convw_d = nc.dram_tensor("convw", [DIL, DC], F32, kind="ExternalInput")
    convb_d = nc.dram_tensor("convb", [DIL, 1], F32, kind="ExternalInput")
    dvec_d = nc.dram_tensor("dvec", [DIL, 1], F32, kind="ExternalInput")
    bdt_d = nc.dram_tensor("bdt", [DIL, 1], F32, kind="ExternalInput")
    identb_d = nc.dram_tensor("identb", [128, 128], BF16, kind="ExternalInput")
    diagd_d = nc.dram_tensor("diagd", [DIL, 128], BF16, kind="ExternalInput")
    out_d = nc.dram_tensor("out_p", [B, L, DM], BF16, kind="ExternalOutput")

    with tile.TileContext(nc) as tc:
        with (
            tc.tile_pool(name="weights", bufs=1) as wp,
            tc.tile_pool(name="resident", bufs=1) as rp,
            tc.tile_pool(name="dram", bufs=1, space="DRAM") as dp,
        ):
            # ---- weights to SBUF ----
            # group 1: what x_branch(0) touches; the rest is issued after its
            # x loads so the first in_proj matmul starts sooner
            win_sb = wp.tile([128, KBLK, 2 * DIL], BF16)
            nc.sync.dma_start(win_sb[:], win_d[:].rearrange("(k p) m -> p k m", p=128))
            wx_sb = wp.tile([128, NBLK, 2 * DS], BF16)
            nc.sync.dma_start(wx_sb[:], wx_d[:].rearrange("(k p) m -> p k m", p=128))
            convw_sb = wp.tile([128, NBLK, DC], F32)
            nc.sync.dma_start(convw_sb[:], convw_d[:].rearrange("(k p) m -> p k m", p=128))
            convb_sb = wp.tile([128, NBLK, 1], F32)
            nc.sync.dma_start(convb_sb[:], convb_d[:].rearrange("(k p) m -> p k m", p=128))
            wout_sb = wp.tile([128, NBLK, DM], BF16)
            wdt_sb = wp.tile([DS, DIL], BF16)
            a_sb = wp.tile([128, NBLK, DS], F32)
            dvec_sb = wp.tile([128, NBLK, 1], F32)
            bdt_sb = wp.tile([128, NBLK, 1], F32)
            identb_sb = wp.tile([128, 128], BF16)
            diagd_sb = wp.tile([128, NBLK, 128], BF16)

            def load_weights_group2():
                nc.sync.dma_start(wout_sb[:], wout_d[:].rearrange("(k p) m -> p k m", p=128))
                nc.sync.dma_start(wdt_sb[:], wdt_d[:])
                nc.sync.dma_start(a_sb[:], a_d[:].rearrange("(k p) m -> p k m", p=128))
                nc.sync.dma_start(dvec_sb[:], dvec_d[:].rearrange("(k p) m -> p k m", p=128))
                nc.sync.dma_start(bdt_sb[:], bdt_d[:].rearrange("(k p) m -> p k m", p=128))
                nc.sync.dma_start(identb_sb[:], identb_d[:])
                nc.sync.dma_start(diagd_sb[:], diagd_d[:].rearrange("(k p) m -> p k m", p=128))

            # ---- resident activations ----
            xcv = [[rp.tile([128, L], BF16, name=f"xcv{b_}{k}", tag=f"xcv{b_}{k}")
                    for k in range(NBLK)] for b_ in range(B)]
            zac = [[rp.tile([128, L], BF16, name=f"zac{b_}{k}", tag=f"zac{b_}{k}")
                    for k in range(NBLK)] for b_ in range(B)]
            # md holds -dt; after the scans it is overwritten in place by the
            # gated ssm output (yin) to save SBUF.
            md = [[rp.tile([128, L], BF16, name=f"md{b_}{k}", tag=f"md{b_}{k}")
                   for k in range(NBLK)] for b_ in range(B)]
            dtin_sb = [rp.tile([DS, L], BF16, name=f"dtin{b_}", tag=f"dtin{b_}")
                       for b_ in range(B)]
            xs_sb = [rp.tile([2 * DS, L], BF16, name=f"xs{b_}", tag=f"xs{b_}")
                     for b_ in range(B)]
            xp = [rp.tile([128, LTA + DC - 1], BF16, name=f"xp{k}", tag=f"xp{k}")
                  for k in range(NBLK)]

            cc_in_dt = [dp.tile([DS, L], BF16, name=f"cc_in_dt{b_}") for b_ in range(B)]
            cc_in_B = [dp.tile([DS, L], BF16, name=f"cc_in_B{b_}") for b_ in range(B)]
            cc_out_dt = [dp.tile([DS, L], BF16, addr_space="Shared",
                                 name=f"cc_out_dt{b_}") for b_ in range(B)]
            cc_out_B = [dp.tile([DS, L], BF16, addr_space="Shared",
                                name=f"cc_out_B{b_}") for b_ in range(B)]

            with (
                tc.tile_pool(name="yps", bufs=1, space="PSUM") as ypsp,
                tc.tile_pool(name="pax", bufs=1) as pax,
                tc.tile_pool(name="pb", bufs=1) as pb,
            ):
                dtpsp_cm = tc.tile_pool(name="dtps", bufs=2, space="PSUM")
                dtpsp = dtpsp_cm.__enter__()
                # ---------------- phase A helpers ----------------
                def x_branch(b_, paps, mid_hook=None):
                    xsx_tiles = []
                    for ch in range(NCH):
                        t0 = ch * LTA
                        xsx = pax.tile([128, KBLK, LTA], BF16, tag="xsx", bufs=4,
                                       name=f"xsx{b_}{ch}")
                        nc.sync.dma_start(
                            xsx[:], x_t[b_].transpose([1, 0, 2])[:, :, t0:t0 + LTA])
                        xsx_tiles.append(xsx)
                    for ch in range(NCH):
                        t0 = ch * LTA
                        xsx = xsx_tiles[ch]
                        for blk in range(NBLK):
                            ps = paps.tile([128, LTA], F32, tag="ps_in",
                                           name=f"psx{b_}{ch}{blk}")
                            for kb in range(KBLK):
                                nc.tensor.matmul(
                                    ps[:],
                                    win_sb[:, kb, blk * 128:(blk + 1) * 128],
                                    xsx[:, kb, :],
                                    start=(kb == 0), stop=(kb == KBLK - 1))
                            # depthwise causal conv, 4 taps, bf16
                            if ch == 0:
                                nc.vector.memset(xp[blk][:, 0:DC - 1], 0.0)
                            else:
                                nc.vector.tensor_copy(
                                    xp[blk][:, 0:DC - 1],
                                    xp[blk][:, LTA:LTA + DC - 1])
                            nc.scalar.copy(xp[blk][:, DC - 1:LTA + DC - 1], ps[:])
                            c0 = pax.tile([128, LTA], BF16, tag="cv0", bufs=2,
                                          name=f"c0{b_}{ch}{blk}")
                            c1 = pax.tile([128, LTA], BF16, tag="cv1", bufs=2,
                                          name=f"c1{b_}{ch}{blk}")
                            c2 = pax.tile([128, LTA], BF16, tag="cv2", bufs=2,
                                          name=f"c2{b_}{ch}{blk}")
                            nc.vector.tensor_scalar_mul(
                                c0[:], xp[blk][:, 0:LTA], convw_sb[:, blk, 0:1])
                            nc.vector.tensor_scalar_mul(
                                c1[:], xp[blk][:, 1:1 + LTA], convw_sb[:, blk, 1:2])
                            nc.vector.tensor_add(c0[:], c0[:], c1[:])
                            nc.vector.tensor_scalar_mul(
                                c1[:], xp[blk][:, 2:2 + LTA], convw_sb[:, blk, 2:3])
                            nc.vector.tensor_scalar_mul(
                                c2[:], xp[blk][:, 3:3 + LTA], convw_sb[:, blk, 3:4])
                            nc.vector.tensor_add(c1[:], c1[:], c2[:])
                            nc.vector.tensor_add(c0[:], c0[:], c1[:])
                            nc.scalar.activation(
                                xcv[b_][blk][:, t0:t0 + LTA], c0[:],
                                AF.Silu, bias=convb_sb[:, blk, :])
                        # x_proj partial for this chunk (borrows a ps_in slot)
                        psx = paps.tile([128, LTA], F32, tag="ps_in",
                                        name=f"psxp{b_}{ch}")
                        for kb in range(NBLK):
                            nc.tensor.matmul(
                                psx[0:2 * DS, :], wx_sb[:, kb, :],
                                xcv[b_][kb][:, t0:t0 + LTA],
                                start=(kb == 0), stop=(kb == NBLK - 1))
                        nc.scalar.copy(xs_sb[b_][:, t0:t0 + LTA], psx[0:2 * DS, :])
                        # ship each chunk's partial immediately: the AllReduce
                        # trigger fires as soon as the LAST chunk lands, ~5-13us
                        # earlier than one bulk copy
                        nc.sync.dma_start(cc_in_dt[b_][:, t0:t0 + LTA],
                                          xs_sb[b_][0:DS, t0:t0 + LTA])
                        nc.sync.dma_start(cc_in_B[b_][:, t0:t0 + LTA],
                                          xs_sb[b_][DS:2 * DS, t0:t0 + LTA])
                        if ch == 2 and mid_hook is not None:
                            mid_hook()
                    return xsx_tiles

                def z_mms(b_, ch, xsz, paps):
                    outs = []
                    for blk in range(NBLK):
                        ps = paps.tile([128, LTA], F32, tag="ps_in",
                                       name=f"psz{b_}{ch}{blk}")
                        for kb in range(KBLK):
                            nc.tensor.matmul(
                                ps[:],
                                win_sb[:, kb, (NBLK + blk) * 128:(NBLK + blk + 1) * 128],
                                xsz[:, kb, :],
                                start=(kb == 0), stop=(kb == KBLK - 1))
                        outs.append((blk, ps))
                    return outs

                def z_silu(b_, ch, outs):
                    t0 = ch * LTA
                    for blk, ps in outs:
                        nc.scalar.activation(
                            zac[b_][blk][:, t0:t0 + LTA], ps[:], AF.Silu)

                def dt_half(b_, blk):
                    # md = -softplus(dt_raw + b_dt) = ln(sigmoid(-(dt_raw + b_dt)))
                    for ch in range(NCH):
                        t0 = ch * LTA
                        psd = dtpsp.tile([128, LTA], F32, tag="psd",
                                         name=f"psd{b_}{ch}{blk}")
                        nc.tensor.matmul(
                            psd[:], wdt_sb[:, blk * 128:(blk + 1) * 128],
                            dtin_sb[b_][:, t0:t0 + LTA],
                            start=True, stop=True)
                        nc.scalar.activation(
                            md[b_][blk][:, t0:t0 + LTA], psd[:],
                            AF.Sigmoid, bias=bdt_sb[:, blk, :], scale=-1.0)
                    nc.scalar.activation(md[b_][blk][:], md[b_][blk][:], AF.Ln)

                def issue_cc_trigger(b_):
                    # dt rows first: the md chain is the critical path
                    nc.gpsimd.collective_compute(
                        "AllReduce", OP.add,
                        ins=[cc_in_dt[b_].opt()], outs=[cc_out_dt[b_].opt()],
                        replica_groups=[list(range(NCORES))])
                    nc.gpsimd.collective_compute(
                        "AllReduce", OP.add,
                        ins=[cc_in_B[b_].opt()], outs=[cc_out_B[b_].opt()],
                        replica_groups=[list(range(NCORES))])

                def issue_bb(b_, blk, p, nsplit=2):
                    # one tile per PAIR of state rows (2p, 2p+1); the split
                    # broadcasts land on several DMA engines in parallel,
                    # cutting the 512KB replication latency
                    bb2 = pb.tile([128, 2, L], BF16, tag="bbn", bufs=2,
                                  name=f"bb{b_}{blk}{p}")
                    h = L // nsplit
                    for j in range(2):
                        for k in range(nsplit):
                            nc.sync.dma_start(
                                bb2[:, j, k * h:(k + 1) * h],
                                cc_out_B[b_][2 * p + j:2 * p + j + 1,
                                             k * h:(k + 1) * h]
                                .broadcast_to([128, h]))
                    return bb2

                # ---------------- phase B helpers ----------------
                _da_zeroed = [0]

                def ssm_block(b_, blk, pre_bb=(), scalar_hook=None):
                    """scans + y accumulation for one (batch, blk). Hooks let
                    us interleave other work into the scalar/gpsimd streams."""
                    dtx = pb.tile([128, L], BF16, tag="dtx", bufs=1,
                                  name=f"dtx{b_}{blk}")
                    nc.vector.tensor_mul(dtx[:], md[b_][blk][:], xcv[b_][blk][:])
                    y_ps = [ypsp.tile([128, 512], F32, tag=f"yps{pt}",
                                      name=f"yps{b_}{blk}{pt}")
                            for pt in range(NPT)]
                    h2 = None
                    for n in range(DS):
                        p, j = n // 2, n % 2
                        if j == 0:
                            bb2 = (pre_bb[p] if p < len(pre_bb)
                                   else issue_bb(b_, blk, p))
                            # da for the PAIR; flat col 2048 (second half t=0)
                            # is a permanent zero -> the merged scan's state
                            # resets exactly at the pair boundary
                            da2 = pb.tile([128, 2, L], F32, tag="dan", bufs=2,
                                          name=f"da{b_}{blk}{p}")
                            if _da_zeroed[0] < 2:
                                nc.vector.memset(da2[:, 1, 0:1], 0.0)
                                _da_zeroed[0] += 1
                            nc.scalar.activation(
                                da2[:, 0, :], md[b_][blk][:], AF.Exp,
                                scale=a_sb[:, blk, n:n + 1])
                            nc.scalar.activation(
                                da2[:, 1, 1:L], md[b_][blk][:, 1:L], AF.Exp,
                                scale=a_sb[:, blk, n + 1:n + 2])
                            u2 = pb.tile([128, 2, L], BF16, tag="un", bufs=2,
                                         name=f"u{b_}{blk}{p}")
                            nc.vector.tensor_mul(
                                u2[:],
                                dtx[:].rearrange("q (o l) -> q o l", o=1)
                                .broadcast_to([128, 2, L]),
                                bb2[:])
                            h2 = pb.tile([128, 2, L], BF16, tag="hn", bufs=2,
                                         name=f"h{b_}{blk}{p}")
                            nc.vector.tensor_tensor_scan(
                                h2[:].rearrange("q o l -> q (o l)"),
                                da2[:].rearrange("q o l -> q (o l)"),
                                u2[:].rearrange("q o l -> q (o l)"),
                                0.0, OP.mult, OP.add)
                        if scalar_hook is not None:
                            scalar_hook(n)
                        for pt in range(NPT):
                            nc.tensor.matmul(
                                y_ps[pt][:], identb_sb[:],
                                h2[:, j, pt * 512:(pt + 1) * 512],
                                start=(n == 0), stop=False)
                    return y_ps

                def gate_block(b_, blk, y_ps, per_pt=None):
                    yin = md[b_][blk]
                    for pt in range(NPT):
                        nc.tensor.matmul(
                            y_ps[pt][:], diagd_sb[:, blk, :],
                            xcv[b_][blk][:, pt * 512:(pt + 1) * 512],
                            start=False, stop=True)
                        nc.vector.tensor_mul(
                            yin[:, pt * 512:(pt + 1) * 512], y_ps[pt][:],
                            zac[b_][blk][:, pt * 512:(pt + 1) * 512])
                        if per_pt is not None:
                            per_pt(pt)
                    return yin

                def out_proj_mt(b_, yins, mt, dmh, psop):
                    ps_o = psop.tile([128, 512], F32, tag="ps_o", bufs=2,
                                     name=f"pso{b_}{mt}{dmh}")
                    for blk in range(NBLK):
                        nc.tensor.matmul(
                            ps_o[:],
                            yins[blk][:, mt * 128:(mt + 1) * 128],
                            wout_sb[:, blk, dmh * 512:(dmh + 1) * 512],
                            start=(blk == 0), stop=(blk == NBLK - 1))
                    osb = pb.tile([128, 512], BF16, tag="osb", bufs=4,
                                  name=f"osb{b_}{mt}{dmh}")
                    nc.scalar.copy(osb[:], ps_o[:])
                    nc.sync.dma_start(
                        out_d[b_, mt * 128:(mt + 1) * 128,
                              dmh * 512:(dmh + 1) * 512],
                        osb[:])

                # ================= issue: phase A =================
                if True:
                    paps_cm = tc.tile_pool(name="paps", bufs=2, space="PSUM")
                    paps = paps_cm.__enter__()
                    xb0_tiles = x_branch(0, paps)
                    load_weights_group2()
                    issue_cc_trigger(0)
                    for ch in range(NCH):
                        z_silu(0, ch, z_mms(0, ch, xb0_tiles[ch], paps))

                    pre_bb00 = []

                    def mid_b1():
                        # dt(b0) threaded mid-x_branch(1): x loads are already
                        # hoisted, so the AllReduce-gated DMAs blocking the
                        # sync queue delay nothing urgent behind them
                        nc.sync.dma_start(dtin_sb[0][:], cc_out_dt[0][:])
                        pre_bb00.append(issue_bb(0, 0, 0, nsplit=4))
                        pre_bb00.append(issue_bb(0, 0, 1))
                        dt_half(0, 0)

                    xb1_tiles = x_branch(1, paps)
                    issue_cc_trigger(1)
                    mid_b1()

                # ================= issue: phase B =================
                if True:
                    # z(b1) and dt(b0,blk1) are issued inside block 0: the
                    # engine-progress semaphores make anything issued before
                    # the ln/exp chain delay it, so all non-critical work goes
                    # after the first exps
                    def sc_hook_00(n):
                        if n == 11:
                            dt_half(0, 1)

                    yps00 = ssm_block(0, 0, pre_bb=pre_bb00,
                                      scalar_hook=sc_hook_00)

                    # dt(b1) blk0: tensor work lands after identity(b0,blk0)
                    nc.sync.dma_start(dtin_sb[1][:], cc_out_dt[1][:])
                    dt_half(1, 0)
                    yin00 = gate_block(0, 0, yps00)

                    # z(b1) hosted in block 1: block 0's broadcast supply is
                    # still catching up from the AllReduce and runs hotter
                    def sc_hook_01(n):
                        if n in (3, 9):
                            for ch in (0, 1) if n == 3 else (2, 3):
                                z_silu(1, ch, z_mms(1, ch, xb1_tiles[ch], paps))

                    yps01 = ssm_block(0, 1, scalar_hook=sc_hook_01)
                    paps_cm.__exit__(None, None, None)
                    dt_half(1, 1)
                    dtpsp_cm.__exit__(None, None, None)
                    psop_cm = tc.tile_pool(name="pso", bufs=4, space="PSUM")
                    psop = psop_cm.__enter__()
                    yin01 = gate_block(0, 1, yps01)

                    # (b1, blk0): interleave out_proj(b0) behind the exps so
                    # the scalar engine drains PSUM without starving the scans
                    op0 = [(mt, dmh) for mt in range(L // 128) for dmh in range(2)]

                    def op_hook_b1(n):
                        for j in range(2):
                            idx = n * 2 + j
                            if idx < len(op0):
                                mt, dmh = op0[idx]
                                out_proj_mt(0, [yin00, yin01], mt, dmh, psop)

                    yps10 = ssm_block(1, 0, scalar_hook=op_hook_b1)
                    yin10 = gate_block(1, 0, yps10)

                    # out_proj(b1): the blk0 half-products for token tiles
                    # mt=4..15 are computed into spare bf16 buffers (dead b0
                    # tiles) while (b1,blk1) scans run, so the tail after the
                    # last scan holds only the blk1 matmuls + cheap adds.
                    obuf_tiles = [xcv[0][0], xcv[0][1], zac[0][0], zac[0][1],
                                  yin00, yin01]

                    def obuf_slot(idx):
                        t = obuf_tiles[(idx - 8) // 4]
                        c = ((idx - 8) % 4) * 512
                        return t[:, c:c + 512]

                    def half0_hook(n):
                        # 2 groups per n covers idx 8..31 over n=0..11
                        for j in range(2):
                            idx = 8 + n * 2 + j
                            if idx < 32:
                                mt, dmh = idx // 2, idx % 2
                                ps_h = psop.tile([128, 512], F32, tag="ps_o",
                                                 bufs=2, name=f"ph{mt}{dmh}")
                                nc.tensor.matmul(
                                    ps_h[:],
                                    yin10[:, mt * 128:(mt + 1) * 128],
                                    wout_sb[:, 0, dmh * 512:(dmh + 1) * 512],
                                    start=True, stop=True)
                                nc.scalar.copy(obuf_slot(idx), ps_h[:])

                    yps11 = ssm_block(1, 1, scalar_hook=half0_hook)

                    # gate blk1 per-pt and chase it with the out_proj(b1) tail
                    def op1_pt(pt):
                        for mt in range(pt * 4, pt * 4 + 4):
                            for dmh in range(2):
                                idx = mt * 2 + dmh
                                if idx < 8:
                                    out_proj_mt(1, [yin10, md[1][1]], mt, dmh,
                                                psop)
                                    continue
                                ps_t = psop.tile([128, 512], F32, tag="ps_o",
                                                 bufs=2, name=f"pt{mt}{dmh}")
                                nc.tensor.matmul(
                                    ps_t[:],
                                    md[1][1][:, mt * 128:(mt + 1) * 128],
                                    wout_sb[:, 1, dmh * 512:(dmh + 1) * 512],
                                    start=True, stop=True)
                                osb = pb.tile([128, 512], BF16, tag="osb", bufs=4,
                                              name=f"osbt{mt}{dmh}")
                                nc.vector.tensor_add(osb[:], ps_t[:],
                                                     obuf_slot(idx))
                                eng = nc.sync if (mt + dmh) % 2 == 0 else nc.scalar
                                eng.dma_start(
                                    out_d[1, mt * 128:(mt + 1) * 128,
                                          dmh * 512:(dmh + 1) * 512],
                                    osb[:])

                    gate_block(1, 1, yps11, per_pt=op1_pt)
                    psop_cm.__exit__(None, None, None)

    nc.compile()
    return nc


_NC_CACHE = {}


def _get_nc():
    if "nc" not in _NC_CACHE:
        _NC_CACHE["nc"] = build_nc()
    return _NC_CACHE["nc"]


def make_in_maps(x, W_in, conv_w, conv_b, W_x, W_dt, b_dt, A_log, D, W_out):
    x = np.asarray(x, np.float32)
    W_in = np.asarray(W_in, np.float32)
    conv_w = np.asarray(conv_w, np.float32)
    conv_b = np.asarray(conv_b, np.float32)
    W_x = np.asarray(W_x, np.float32)
    W_dt = np.asarray(W_dt, np.float32)
    b_dt = np.asarray(b_dt, np.float32)
    A_log = np.asarray(A_log, np.float32)
    D = np.asarray(D, np.float32)
    W_out = np.asarray(W_out, np.float32)

    xt = np.ascontiguousarray(x.transpose(0, 2, 1)).reshape(B, KBLK, 128, L).astype(ml_dtypes.bfloat16)
    A = np.exp(A_log)  # |A|; device uses a = -|A|, md = +dt

    in_maps = []
    for c in range(NCORES):
        lo = c * DIL
        sl = slice(lo, lo + DIL)
        in_maps.append({
            "x_t": xt,
            "win": np.ascontiguousarray(
                np.concatenate([W_in[:, sl], W_in[:, DI + lo:DI + lo + DIL]],
                               axis=1)).astype(ml_dtypes.bfloat16),
            "wout": np.ascontiguousarray(W_out[sl]).astype(ml_dtypes.bfloat16),
            "wx": np.ascontiguousarray(
                np.concatenate([W_x[sl, :DS], -W_x[sl, DS:]], axis=1)
            ).astype(ml_dtypes.bfloat16),
            "wdt": np.ascontiguousarray(W_dt[:, sl]).astype(ml_dtypes.bfloat16),
            "a": np.ascontiguousarray(A[sl]),
            "convw": np.ascontiguousarray(conv_w[sl]),
            "convb": np.ascontiguousarray(conv_b[sl, None]),
            "dvec": np.ascontiguousarray(D[sl, None]),
            "bdt": np.ascontiguousarray(-b_dt[sl, None]),
            "identb": np.eye(128, dtype=ml_dtypes.bfloat16),
            "diagd": np.stack([np.diag(D[lo + k * 128:lo + (k + 1) * 128])
                               for k in range(NBLK)]).reshape(DIL, 128)
                       .astype(ml_dtypes.bfloat16),
        })
    return in_maps


def kernel(**inputs):
    nc = _get_nc()
    in_maps = make_in_maps(**inputs)
    res = run_bass_kernel_spmd(nc, in_maps, list(range(NCORES)))
    out = np.zeros((B, L, DM), np.float32)
    for c in range(NCORES):
        out += np.asarray(res.results[c]["out_p"], dtype=np.float32)
    return out


# revision 39
# speedup vs baseline: 1.0264x; 1.0146x over previous
"""Mamba block (MockMambaBlock) on 8 Trainium2 NeuronCores.

Sharding: tensor-parallel over d_inner (8 x 256 channels), both batches on
every core. The x_proj/dt_proj contraction over d_inner is completed with an
on-device AllReduce of the small (32, L) partial per batch; out_proj
row-partials are summed on the host (the gather step).

Schedule: phase A computes the in_proj x-half (conv + x_proj) of batch 0
first so its AllReduce fires early; the z-half and batch 1 follow. The SSM
scans (the DVE wall: 2 cycles/element regardless of dtype) start as soon as
md(b0) lands and overlap the rest of phase A. The u = dtx*B_n multiplies stay on
the DVE (GpSimd shares SBUF ports with the DVE, so offloading them slows the
scans more than it saves); exp/silu/sigmoid and the PSUM->SBUF drains run on
the scalar engine, interleaved so the scan supply chain never starves. out_proj is emitted per 128-token tile right behind the
gating to keep the tail short. Issue order doubles as per-engine program
order, so hooks thread low-priority work into the busy streams.
"""

import sys

sys.path.insert(0, "/opt/trn_rl_repo")

import numpy as np
import ml_dtypes

import concourse.bass as bass
import concourse.bacc as bacc
import concourse.mybir as mybir
import concourse.tile as tile
from concourse.bass_utils import run_bass_kernel_spmd

F32 = mybir.dt.float32
BF16 = mybir.dt.bfloat16
AF = mybir.ActivationFunctionType
OP = mybir.AluOpType

B, L, DM, DI, DS, DC = 2, 2048, 1024, 2048, 16, 4
NCORES = 8
DIL = DI // NCORES          # 256 channels per core
NBLK = DIL // 128           # 2 partition blocks of channels
KBLK = DM // 128            # 8 contraction blocks for in_proj
LTA = 512                   # token chunk
NCH = L // LTA              # 4 chunks
NPT = L // 512              # psum tiles per row


def build_nc():
    nc = bacc.Bacc()

    x_t = nc.dram_tensor("x_t", [B, KBLK, 128, L], BF16, kind="ExternalInput")
    win_d = nc.dram_tensor("win", [DM, 2 * DIL], BF16, kind="ExternalInput")
    wout_d = nc.dram_tensor("wout", [DIL, DM], BF16, kind="ExternalInput")
    wx_d = nc.dram_tensor("wx", [DIL, 2 * DS], BF16, kind="ExternalInput")
    wdt_d = nc.dram_tensor("wdt", [DS, DIL], BF16, kind="ExternalInput")
    a_d = nc.dram_tensor("a", [DIL, DS], F32, kind="ExternalInput")
    convw_d = nc.dram_tensor("convw", [DIL, DC], F32, kind="ExternalInput")
    convb_d = nc.dram_tensor("convb", [DIL, 1], F32, kind="ExternalInput")
    dvec_d = nc.dram_tensor("dvec", [DIL, 1], F32, kind="ExternalInput")
    bdt_d = nc.dram_tensor("bdt", [DIL, 1], F32, kind="ExternalInput")
    identb_d = nc.dram_tensor("identb", [128, 128], BF16, kind="ExternalInput")
    diagd_d = nc.dram_tensor("diagd", [DIL, 128], BF16, kind="ExternalInput")
    out_d = nc.dram_tensor("out_p", [B, L, DM], BF16, kind="ExternalOutput")

    with tile.TileContext(nc) as tc:
        with (
            tc.tile_pool(name="weights", bufs=1) as wp,
            tc.tile_pool(name="resident", bufs=1) as rp,
            tc.tile_pool(name="dram", bufs=1, space="DRAM") as dp,
        ):
            # ---- weights to SBUF ----
            # group 1: what x_branch(0) touches; the rest is issued after its
            # x loads so the first in_proj matmul starts sooner
            win_sb = wp.tile([128, KBLK, 2 * DIL], BF16)
            nc.sync.dma_start(win_sb[:], win_d[:].rearrange("(k p) m -> p k m", p=128))
            wx_sb = wp.tile([128, NBLK, 2 * DS], BF16)
            nc.sync.dma_start(wx_sb[:], wx_d[:].rearrange("(k p) m -> p k m", p=128))
            convw_sb = wp.tile([128, NBLK, DC], F32)
            nc.sync.dma_start(convw_sb[:], convw_d[:].rearrange("(k p) m -> p k m", p=128))
            convb_sb = wp.tile([128, NBLK, 1], F32)
            nc.sync.dma_start(convb_sb[:], convb_d[:].rearrange("(k p) m -> p k m", p=128))
            wout_sb = wp.tile([128, NBLK, DM], BF16)
            wdt_sb = wp.tile([DS, DIL], BF16)
            a_sb = wp.tile([128, NBLK, DS], F32)
            dvec_sb = wp.tile([128, NBLK, 1], F32)
            bdt_sb = wp.tile([128, NBLK, 1], F32)
            identb_sb = wp.tile([128, 128], BF16)
            diagd_sb = wp.tile([128, NBLK, 128], BF16)

            def load_weights_group2():
                nc.sync.dma_start(wout_sb[:], wout_d[:].rearrange("(k p) m -> p k m", p=128))
                nc.sync.dma_start(wdt_sb[:], wdt_d[:])
                nc.sync.dma_start(a_sb[:], a_d[:].rearrange("(k p) m -> p k m", p=128))
                nc.sync.dma_start(dvec_sb[:], dvec_d[:].rearrange("(k p) m -> p k m", p=128))
                nc.sync.dma_start(bdt_sb[:], bdt_d[:].rearrange("(k p) m -> p k m", p=128))
                nc.sync.dma_start(identb_sb[:], identb_d[:])
                nc.sync.dma_start(diagd_sb[:], diagd_d[:].rearrange("(k p) m -> p k m", p=128))

            # ---- resident activations ----
            xcv = [[rp.tile([128, L], BF16, name=f"xcv{b_}{k}", tag=f"xcv{b_}{k}")
                    for k in range(NBLK)] for b_ in range(B)]
            zac = [[rp.tile([128, L], BF16, name=f"zac{b_}{k}", tag=f"zac{b_}{k}")
                    for k in range(NBLK)] for b_ in range(B)]
            # md holds -dt; after the scans it is overwritten in place by the
            # gated ssm output (yin) to save SBUF.
            md = [[rp.tile([128, L], BF16, name=f"md{b_}{k}", tag=f"md{b_}{k}")
                   for k in range(NBLK)] for b_ in range(B)]
            dtin_sb = [rp.tile([DS, L], BF16, name=f"dtin{b_}", tag=f"dtin{b_}")
                       for b_ in range(B)]
            xs_sb = [rp.tile([2 * DS, L], BF16, name=f"xs{b_}", tag=f"xs{b_}")
                     for b_ in range(B)]
            xp = [rp.tile([128, LTA + DC - 1], BF16, name=f"xp{k}", tag=f"xp{k}")
                  for k in range(NBLK)]

            cc_in_dt = [dp.tile([DS, L], BF16, name=f"cc_in_dt{b_}") for b_ in range(B)]
            cc_in_B = [dp.tile([DS, L], BF16, name=f"cc_in_B{b_}") for b_ in range(B)]
            cc_out_dt = [dp.tile([DS, L], BF16, addr_space="Shared",
                                 name=f"cc_out_dt{b_}") for b_ in range(B)]
            cc_out_B = [dp.tile([DS, L], BF16, addr_space="Shared",
                                name=f"cc_out_B{b_}") for b_ in range(B)]

            with (
                tc.tile_pool(name="yps", bufs=1, space="PSUM") as ypsp,
                tc.tile_pool(name="pax", bufs=1) as pax,
                tc.tile_pool(name="pb", bufs=1) as pb,
            ):
                dtpsp_cm = tc.tile_pool(name="dtps", bufs=2, space="PSUM")
                dtpsp = dtpsp_cm.__enter__()
                # ---------------- phase A helpers ----------------
                def x_branch(b_, paps, mid_hook=None):
                    xsx_tiles = []
                    for ch in range(NCH):
                        t0 = ch * LTA
                        xsx = pax.tile([128, KBLK, LTA], BF16, tag="xsx", bufs=4,
                                       name=f"xsx{b_}{ch}")
                        nc.sync.dma_start(
                            xsx[:], x_t[b_].transpose([1, 0, 2])[:, :, t0:t0 + LTA])
                        xsx_tiles.append(xsx)
                    for ch in range(NCH):
                        t0 = ch * LTA
                        xsx = xsx_tiles[ch]
                        for blk in range(NBLK):
                            ps = paps.tile([128, LTA], F32, tag="ps_in",
                                           name=f"psx{b_}{ch}{blk}")
                            for kb in range(KBLK):
                                nc.tensor.matmul(
                                    ps[:],
                                    win_sb[:, kb, blk * 128:(blk + 1) * 128],
                                    xsx[:, kb, :],
                                    start=(kb == 0), stop=(kb == KBLK - 1))
                            # depthwise causal conv, 4 taps, bf16
                            if ch == 0:
                                nc.vector.memset(xp[blk][:, 0:DC - 1], 0.0)
                            else:
                                nc.vector.tensor_copy(
                                    xp[blk][:, 0:DC - 1],
                                    xp[blk][:, LTA:LTA + DC - 1])
                            nc.scalar.copy(xp[blk][:, DC - 1:LTA + DC - 1], ps[:])
                            c0 = pax.tile([128, LTA], BF16, tag="cv0", bufs=2,
                                          name=f"c0{b_}{ch}{blk}")
                            c1 = pax.tile([128, LTA], BF16, tag="cv1", bufs=2,
                                          name=f"c1{b_}{ch}{blk}")
                            c2 = pax.tile([128, LTA], BF16, tag="cv2", bufs=2,
                                          name=f"c2{b_}{ch}{blk}")
                            nc.vector.tensor_scalar_mul(
                                c0[:], xp[blk][:, 0:LTA], convw_sb[:, blk, 0:1])
                            nc.vector.tensor_scalar_mul(
                                c1[:], xp[blk][:, 1:1 + LTA], convw_sb[:, blk, 1:2])
                            nc.vector.tensor_add(c0[:], c0[:], c1[:])
                            nc.vector.tensor_scalar_mul(
                                c1[:], xp[blk][:, 2:2 + LTA], convw_sb[:, blk, 2:3])
                            nc.vector.tensor_scalar_mul(
                                c2[:], xp[blk][:, 3:3 + LTA], convw_sb[:, blk, 3:4])
                            nc.vector.tensor_add(c1[:], c1[:], c2[:])
                            nc.vector.tensor_add(c0[:], c0[:], c1[:])
                            nc.scalar.activation(
                                xcv[b_][blk][:, t0:t0 + LTA], c0[:],
                                AF.Silu, bias=convb_sb[:, blk, :])
                        # x_proj partial for this chunk (borrows a ps_in slot)
                        psx = paps.tile([128, LTA], F32, tag="ps_in",
                                        name=f"psxp{b_}{ch}")
                        for kb in range(NBLK):
                            nc.tensor.matmul(
                                psx[0:2 * DS, :], wx_sb[:, kb, :],
                                xcv[b_][kb][:, t0:t0 + LTA],
                                start=(kb == 0), stop=(kb == NBLK - 1))
                        nc.scalar.copy(xs_sb[b_][:, t0:t0 + LTA], psx[0:2 * DS, :])
                        # ship each chunk's partial immediately: the AllReduce
                        # trigger fires as soon as the LAST chunk lands, ~5-13us
                        # earlier than one bulk copy
                        nc.sync.dma_start(cc_in_dt[b_][:, t0:t0 + LTA],
                                          xs_sb[b_][0:DS, t0:t0 + LTA])
                        nc.sync.dma_start(cc_in_B[b_][:, t0:t0 + LTA],
                                          xs_sb[b_][DS:2 * DS, t0:t0 + LTA])
                        if ch == 2 and mid_hook is not None:
                            mid_hook()
                    return xsx_tiles

                def z_mms(b_, ch, xsz, paps):
                    outs = []
                    for blk in range(NBLK):
                        ps = paps.tile([128, LTA], F32, tag="ps_in",
                                       name=f"psz{b_}{ch}{blk}")
                        for kb in range(KBLK):
                            nc.tensor.matmul(
                                ps[:],
                                win_sb[:, kb, (NBLK + blk) * 128:(NBLK + blk + 1) * 128],
                                xsz[:, kb, :],
                                start=(kb == 0), stop=(kb == KBLK - 1))
                        outs.append((blk, ps))
                    return outs

                def z_silu(b_, ch, outs):
                    t0 = ch * LTA
                    for blk, ps in outs:
                        nc.scalar.activation(
                            zac[b_][blk][:, t0:t0 + LTA], ps[:], AF.Silu)

                def dt_half(b_, blk):
                    # md = -softplus(dt_raw + b_dt) = ln(sigmoid(-(dt_raw + b_dt)))
                    for ch in range(NCH):
                        t0 = ch * LTA
                        psd = dtpsp.tile([128, LTA], F32, tag="psd",
                                         name=f"psd{b_}{ch}{blk}")
                        nc.tensor.matmul(
                            psd[:], wdt_sb[:, blk * 128:(blk + 1) * 128],
                            dtin_sb[b_][:, t0:t0 + LTA],
                            start=True, stop=True)
                        nc.scalar.activation(
                            md[b_][blk][:, t0:t0 + LTA], psd[:],
                            AF.Sigmoid, bias=bdt_sb[:, blk, :], scale=-1.0)
                    nc.scalar.activation(md[b_][blk][:], md[b_][blk][:], AF.Ln)

                def issue_cc_trigger(b_):
                    # dt rows first: the md chain is the critical path
                    nc.gpsimd.collective_compute(
                        "AllReduce", OP.add,
                        ins=[cc_in_dt[b_].opt()], outs=[cc_out_dt[b_].opt()],
                        replica_groups=[list(range(NCORES))])
                    nc.gpsimd.collective_compute(
                        "AllReduce", OP.add,
                        ins=[cc_in_B[b_].opt()], outs=[cc_out_B[b_].opt()],
                        replica_groups=[list(range(NCORES))])

                def issue_bb(b_, blk, p, nsplit=2):
                    # one tile per PAIR of state rows (2p, 2p+1); the split
                    # broadcasts land on several DMA engines in parallel,
                    # cutting the 512KB replication latency
                    bb2 = pb.tile([128, 2, L], BF16, tag="bbn", bufs=2,
                                  name=f"bb{b_}{blk}{p}")
                    h = L // nsplit
                    for j in range(2):
                        for k in range(nsplit):
                            nc.sync.dma_start(
                                bb2[:, j, k * h:(k + 1) * h],
                                cc_out_B[b_][2 * p + j:2 * p + j + 1,
                                             k * h:(k + 1) * h]
                                .broadcast_to([128, h]))
                    return bb2

                # ---------------- phase B helpers ----------------
                _da_zeroed = [0]

                def ssm_block(b_, blk, pre_bb=(), scalar_hook=None):
                    """scans + y accumulation for one (batch, blk). Hooks let
                    us interleave other work into the scalar/gpsimd streams."""
                    dtx = pb.tile([128, L], BF16, tag="dtx", bufs=1,
                                  name=f"dtx{b_}{blk}")
                    nc.vector.tensor_mul(dtx[:], md[b_][blk][:], xcv[b_][blk][:])
                    y_ps = [ypsp.tile([128, 512], F32, tag=f"yps{pt}",
                                      name=f"yps{b_}{blk}{pt}")
                            for pt in range(NPT)]
                    h2 = None
                    for n in range(DS):
                        p, j = n // 2, n % 2
                        if j == 0:
                            bb2 = (pre_bb[p] if p < len(pre_bb)
                                   else issue_bb(b_, blk, p))
                            # da for the PAIR; flat col 2048 (second half t=0)
                            # is a permanent zero -> the merged scan's state
                            # resets exactly at the pair boundary
                            da2 = pb.tile([128, 2, L], F32, tag="dan", bufs=2,
                                          name=f"da{b_}{blk}{p}")
                            if _da_zeroed[0] < 2:
                                nc.vector.memset(da2[:, 1, 0:1], 0.0)
                                _da_zeroed[0] += 1
                            nc.scalar.activation(
                                da2[:, 0, :], md[b_][blk][:], AF.Exp,
                                scale=a_sb[:, blk, n:n + 1])
                            nc.scalar.activation(
                                da2[:, 1, 1:L], md[b_][blk][:, 1:L], AF.Exp,
                                scale=a_sb[:, blk, n + 1:n + 2])
                            u2 = pb.tile([128, 2, L], BF16, tag="un", bufs=2,
                                         name=f"u{b_}{blk}{p}")
                            nc.vector.tensor_mul(
                                u2[:],
                                dtx[:].rearrange("q (o l) -> q o l", o=1)
                                .broadcast_to([128, 2, L]),
                                bb2[:])
                            h2 = pb.tile([128, 2, L], BF16, tag="hn", bufs=2,
                                         name=f"h{b_}{blk}{p}")
                            nc.vector.tensor_tensor_scan(
                                h2[:].rearrange("q o l -> q (o l)"),
                                da2[:].rearrange("q o l -> q (o l)"),
                                u2[:].rearrange("q o l -> q (o l)"),
                                0.0, OP.mult, OP.add)
                        if scalar_hook is not None:
                            scalar_hook(n)
                        for pt in range(NPT):
                            nc.tensor.matmul(
                                y_ps[pt][:], identb_sb[:],
                                h2[:, j, pt * 512:(pt + 1) * 512],
                                start=(n == 0), stop=False)
                    return y_ps

                def gate_block(b_, blk, y_ps, per_pt=None):
                    yin = md[b_][blk]
                    for pt in range(NPT):
                        nc.tensor.matmul(
                            y_ps[pt][:], diagd_sb[:, blk, :],
                            xcv[b_][blk][:, pt * 512:(pt + 1) * 512],
                            start=False, stop=True)
                        nc.vector.tensor_mul(
                            yin[:, pt * 512:(pt + 1) * 512], y_ps[pt][:],
                            zac[b_][blk][:, pt * 512:(pt + 1) * 512])
                        if per_pt is not None:
                            per_pt(pt)
                    return yin

                def out_proj_mt(b_, yins, mt, dmh, psop):
                    ps_o = psop.tile([128, 512], F32, tag="ps_o", bufs=2,
                                     name=f"pso{b_}{mt}{dmh}")
                    for blk in range(NBLK):
                        nc.tensor.matmul(
                            ps_o[:],
                            yins[blk][:, mt * 128:(mt + 1) * 128],
                            wout_sb[:, blk, dmh * 512:(dmh + 1) * 512],
                            start=(blk == 0), stop=(blk == NBLK - 1))
                    osb = pb.tile([128, 512], BF16, tag="osb", bufs=4,
                                  name=f"osb{b_}{mt}{dmh}")
                    nc.scalar.copy(osb[:], ps_o[:])
                    nc.sync.dma_start(
                        out_d[b_, mt * 128:(mt + 1) * 128,
                              dmh * 512:(dmh + 1) * 512],
                        osb[:])

                # ================= issue: phase A =================
                if True:
                    paps_cm = tc.tile_pool(name="paps", bufs=2, space="PSUM")
                    paps = paps_cm.__enter__()
                    xb0_tiles = x_branch(0, paps)
                    load_weights_group2()
                    issue_cc_trigger(0)
                    for ch in range(NCH):
                        z_silu(0, ch, z_mms(0, ch, xb0_tiles[ch], paps))

                    pre_bb00 = []

                    def mid_b1():
                        # dt(b0) threaded mid-x_branch(1): x loads are already
                        # hoisted, so the AllReduce-gated DMAs blocking the
                        # sync queue delay nothing urgent behind them
                        nc.sync.dma_start(dtin_sb[0][:], cc_out_dt[0][:])
                        pre_bb00.append(issue_bb(0, 0, 0, nsplit=4))
                        pre_bb00.append(issue_bb(0, 0, 1, nsplit=4))
                        dt_half(0, 0)

                    xb1_tiles = x_branch(1, paps)
                    issue_cc_trigger(1)
                    mid_b1()

                # ================= issue: phase B =================
                if True:
                    # z(b1) and dt(b0,blk1) are issued inside block 0: the
                    # engine-progress semaphores make anything issued before
                    # the ln/exp chain delay it, so all non-critical work goes
                    # after the first exps
                    def sc_hook_00(n):
                        if n == 11:
                            dt_half(0, 1)

                    yps00 = ssm_block(0, 0, pre_bb=pre_bb00,
                                      scalar_hook=sc_hook_00)

                    # dt(b1) blk0: tensor work lands after identity(b0,blk0)
                    nc.sync.dma_start(dtin_sb[1][:], cc_out_dt[1][:])
                    dt_half(1, 0)
                    yin00 = gate_block(0, 0, yps00)

                    # z(b1) hosted in block 1: block 0's broadcast supply is
                    # still catching up from the AllReduce and runs hotter
                    def sc_hook_01(n):
                        if n in (3, 9):
                            for ch in (0, 1) if n == 3 else (2, 3):
                                z_silu(1, ch, z_mms(1, ch, xb1_tiles[ch], paps))

                    yps01 = ssm_block(0, 1, scalar_hook=sc_hook_01)
                    paps_cm.__exit__(None, None, None)
                    dt_half(1, 1)
                    dtpsp_cm.__exit__(None, None, None)
                    psop_cm = tc.tile_pool(name="pso", bufs=4, space="PSUM")
                    psop = psop_cm.__enter__()
                    yin01 = gate_block(0, 1, yps01)

                    # (b1, blk0): interleave out_proj(b0) behind the exps so
                    # the scalar engine drains PSUM without starving the scans
                    op0 = [(mt, dmh) for mt in range(L // 128) for dmh in range(2)]

                    def op_hook_b1(n):
                        for j in range(2):
                            idx = n * 2 + j
                            if idx < len(op0):
                                mt, dmh = op0[idx]
                                out_proj_mt(0, [yin00, yin01], mt, dmh, psop)

                    yps10 = ssm_block(1, 0, scalar_hook=op_hook_b1)
                    yin10 = gate_block(1, 0, yps10)

                    # out_proj(b1): the blk0 half-products for token tiles
                    # mt=4..15 are computed into spare bf16 buffers (dead b0
                    # tiles) while (b1,blk1) scans run, so the tail after the
                    # last scan holds only the blk1 matmuls + cheap adds.
                    obuf_tiles = [xcv[0][0], xcv[0][1], zac[0][0], zac[0][1],
                                  yin00, yin01]

                    def obuf_slot(idx):
                        t = obuf_tiles[(idx - 8) // 4]
                        c = ((idx - 8) % 4) * 512
                        return t[:, c:c + 512]

                    def half0_hook(n):
                        # 2 groups per n covers idx 8..31 over n=0..11
                        for j in range(2):
                            idx = 8 + n * 2 + j
                            if idx < 32:
                                mt, dmh = idx // 2, idx % 2
                                ps_h = psop.tile([128, 512], F32, tag="ps_o",
                                                 bufs=2, name=f"ph{mt}{dmh}")
                                nc.tensor.matmul(
                                    ps_h[:],
                                    yin10[:, mt * 128:(mt + 1) * 128],
                                    wout_sb[:, 0, dmh * 512:(dmh + 1) * 512],
                                    start=True, stop=True)
                                nc.scalar.copy(obuf_slot(idx), ps_h[:])

                    yps11 = ssm_block(1, 1, scalar_hook=half0_hook)

                    # gate blk1 per-pt and chase it with the out_proj(b1) tail
                    def op1_pt(pt):
                        for mt in range(pt * 4, pt * 4 + 4):
                            for dmh in range(2):
                                idx = mt * 2 + dmh
                                if idx < 8:
                                    out_proj_mt(1, [yin10, md[1][1]], mt, dmh,
                                                psop)
                                    continue
                                ps_t = psop.tile([128, 512], F32, tag="ps_o",
                                                 bufs=2, name=f"pt{mt}{dmh}")
                                nc.tensor.matmul(
                                    ps_t[:],
                                    md[1][1][:, mt * 128:(mt + 1) * 128],
                                    wout_sb[:, 1, dmh * 512:(dmh + 1) * 512],
                                    start=True, stop=True)
                                osb = pb.tile([128, 512], BF16, tag="osb", bufs=4,
                                              name=f"osbt{mt}{dmh}")
                                nc.vector.tensor_add(osb[:], ps_t[:],
                                                     obuf_slot(idx))
                                eng = nc.sync if (mt + dmh) % 2 == 0 else nc.scalar
                                eng.dma_start(
                                    out_d[1, mt * 128:(mt + 1) * 128,
                                          dmh * 512:(dmh + 1) * 512],
                                    osb[:])

                    gate_block(1, 1, yps11, per_pt=op1_pt)
                    psop_cm.__exit__(None, None, None)

    nc.compile()
    return nc


_NC_CACHE = {}


def _get_nc():
    if "nc" not in _NC_CACHE:
        _NC_CACHE["nc"] = build_nc()
    return _NC_CACHE["nc"]


def make_in_maps(x, W_in, conv_w, conv_b, W_x, W_dt, b_dt, A_log, D, W_out):
    x = np.asarray(x, np.float32)
    W_in = np.asarray(W_in, np.float32)
    conv_w = np.asarray(conv_w, np.float32)
    conv_b = np.asarray(conv_b, np.float32)
    W_x = np.asarray(W_x, np.float32)
    W_dt = np.asarray(W_dt, np.float32)
    b_dt = np.asarray(b_dt, np.float32)
    A_log = np.asarray(A_log, np.float32)
    D = np.asarray(D, np.float32)
    W_out = np.asarray(W_out, np.float32)

    xt = np.ascontiguousarray(x.transpose(0, 2, 1)).reshape(B, KBLK, 128, L).astype(ml_dtypes.bfloat16)
    A = np.exp(A_log)  # |A|; device uses a = -|A|, md = +dt

    in_maps = []
    for c in range(NCORES):
        lo = c * DIL
        sl = slice(lo, lo + DIL)
        in_maps.append({
            "x_t": xt,
            "win": np.ascontiguousarray(
                np.concatenate([W_in[:, sl], W_in[:, DI + lo:DI + lo + DIL]],
                               axis=1)).astype(ml_dtypes.bfloat16),
            "wout": np.ascontiguousarray(W_out[sl]).astype(ml_dtypes.bfloat16),
            "wx": np.ascontiguousarray(
                np.concatenate([W_x[sl, :DS], -W_x[sl, DS:]], axis=1)
            ).astype(ml_dtypes.bfloat16),
            "wdt": np.ascontiguousarray(W_dt[:, sl]).astype(ml_dtypes.bfloat16),
            "a": np.ascontiguousarray(A[sl]),
            "convw": np.ascontiguousarray(conv_w[sl]),
            "convb": np.ascontiguousarray(conv_b[sl, None]),
            "dvec": np.ascontiguousarray(D[sl, None]),
            "bdt": np.ascontiguousarray(-b_dt[sl, None]),
            "identb": np.eye(128, dtype=ml_dtypes.bfloat16),
            "diagd": np.stack([np.diag(D[lo + k * 128:lo + (k + 1) * 128])
                               for k in range(NBLK)]).reshape(DIL, 128)
                       .astype(ml_dtypes.bfloat16),
        })
    return in_maps


def kernel(**inputs):
    nc = _get_nc()
    in_maps = make_in_maps(**inputs)
    res = run_bass_kernel_spmd(nc, in_maps, list(range(NCORES)))
    out = np.zeros((B, L, DM), np.float32)
    for c in range(NCORES):
        out += np.asarray(res.results[c]["out_p"], dtype=np.float32)
    return out
